# revision 1
# baseline (speedup 1.0000x reference)
"""Bass/Trainium2 SPMD kernel for a 2-layer GCN encoder.

Math (per reference):
    src/dst = edges + self-loops
    deg[v]  = #edges with dst==v (incl self-loop);  dinv = 1/sqrt(deg)
    layer(x, W, b): out[d] = dinv[d] * sum_{e: dst_e==d} dinv[src_e] * (x@W)[src_e] + b
    y = layer1(sigmoid(layer0(x, W0, b0)), W1, b1)

Distribution: nodes are sharded contiguously across 8 cores (6250 each).
Edges are owned by the destination core.  Each core:
  1. GEMM on its x rows, pre-scales rows by dinv (so the per-edge weight
     dinv[src]*dinv[dst] factorizes into a row pre-scale and an output
     post-scale), AllGathers the scaled features.
  2. For each 128-row destination block, gathers the source rows of its
     edges (dma_gather, int16 indices => the node table is split in two
     halves), builds one-hot scatter matrices on the vector engine
     (iota == slot), and scatter-adds via TensorE matmuls accumulating in
     PSUM.  Bias enters as a rank-1 matmul (sqrt(deg) x b), so the final
     PSUM->SBUF copy can apply the dinv post-scale (and sigmoid) in one
     ScalarE activation.
"""

import math

import numpy as np

import concourse.bacc as bacc
import concourse.bass as bass
import concourse.mybir as mybir
import concourse.tile as tile
from concourse.bass_utils import run_bass_kernel_spmd

P = 128
F32 = mybir.dt.float32
BF16 = mybir.dt.bfloat16
I16 = mybir.dt.int16

# Full-problem constants
N_NODES = 50000
N_CORES = 8
F0, F1, F2 = 128, 128, 64
GROUP_BLOCKS = 3  # dst blocks per dma_gather batch
# Per-(block,half) edge-segment alignment. Must stay 128: sub-128 matmul
# pieces with different base partitions back-to-back hard-crash the PE
# (verified on HW: K64@p0 directly followed by K64@p64 aborts the NEFF).
SEG_ALIGN = 128


def _round_up(x, m):
    return (x + m - 1) // m * m


class Plan:
    """Compile-time schedule, identical across cores (SPMD)."""

    def __init__(self, n_nodes, n_cores, gb):
        assert n_nodes % n_cores == 0
        self.n_nodes = n_nodes
        self.n_cores = n_cores
        self.npc = n_nodes // n_cores
        self.nblk = math.ceil(self.npc / P)
        self.hb = (n_nodes + 1) // 2  # half boundary for int16 gather indices
        assert self.hb <= 32768
        self.gb = gb
        self.groups = [
            list(range(i, min(i + gb, self.nblk))) for i in range(0, self.nblk, gb)
        ]
        self.g_of = {}
        for gi, blocks in enumerate(self.groups):
            for b in blocks:
                self.g_of[b] = gi
        # filled by finalize(): per-(blk, half) uniform padded sizes
        self.SZ = None  # [nblk, 2] int, multiples of SEG_ALIGN
        self.seg_off = {}  # (b, h) -> edge offset within its gather
        self.seg_idx16 = {}  # (g_idx, h) -> int16-column base of that gather
        self.seg_colbase = {}  # (g_idx, h) -> global chunk-column base
        self.gather_nid = {}  # (g_idx, h) -> num idxs
        self.ncols = 0
        self.tot16 = 0

    def finalize(self, sz):
        self.SZ = sz
        col = 0
        i16 = 0
        for gi, blocks in enumerate(self.groups):
            for h in (0, 1):
                off = 0
                for b in blocks:
                    self.seg_off[(b, h)] = off
                    off += int(self.SZ[b, h])
                nid = off
                self.gather_nid[(gi, h)] = nid
                self.seg_idx16[(gi, h)] = i16
                self.seg_colbase[(gi, h)] = col
                col += (nid + P - 1) // P
                i16 += nid // 16
        self.ncols = col
        self.tot16 = i16


def _build_metadata(edges, n_nodes, n_cores, gb=GROUP_BLOCKS):
    """Host-side integer preprocessing: shard + sort edges, build gather
    indices / slot vectors / degree tables.  Returns (plan, per_core dict)."""
    plan = Plan(n_nodes, n_cores, gb)
    npc, nblk, hb = plan.npc, plan.nblk, plan.hb

    loop = np.arange(n_nodes, dtype=np.int64)
    src = np.concatenate([np.asarray(edges[0], dtype=np.int64), loop])
    dst = np.concatenate([np.asarray(edges[1], dtype=np.int64), loop])
    deg = np.bincount(dst, minlength=n_nodes).astype(np.float32)

    owner = dst // npc
    ldst = dst % npc
    blk = ldst // P
    slot = (ldst % P).astype(np.float32)
    half = (src >= hb).astype(np.int64)
    cell = ((owner * nblk) + blk) * 2 + half
    order = np.lexsort((src, cell))
    cell_s = cell[order]
    src_s = src[order]
    slot_s = slot[order]

    ncells = n_cores * nblk * 2
    counts = np.bincount(cell_s, minlength=ncells).reshape(n_cores, nblk, 2)
    starts = np.concatenate([[0], np.cumsum(counts.reshape(-1))])[:-1].reshape(
        n_cores, nblk, 2
    )
    sz = np.maximum(counts.max(axis=0), 0)
    sz = (np.ceil(sz / SEG_ALIGN).astype(np.int64)) * SEG_ALIGN  # [nblk, 2]
    plan.finalize(sz)

    ncols = plan.ncols
    tot16 = plan.tot16

    per_core = []
    for c in range(n_cores):
        idx16 = np.zeros((16, tot16), np.int16)
        slots_t = np.full((P, ncols), -1.0, np.float32)
        for gi, blocks in enumerate(plan.groups):
            for h in (0, 1):
                i16b = plan.seg_idx16[(gi, h)] * 16
                colb = plan.seg_colbase[(gi, h)] * P
                for b in blocks:
                    n = int(counts[c, b, h])
                    s0 = int(starts[c, b, h])
                    if n:
                        j = plan.seg_off[(b, h)] + np.arange(n)
                        seg_src = (src_s[s0 : s0 + n] - h * hb).astype(np.int16)
                        ji = i16b + j
                        idx16[ji % 16, ji // 16] = seg_src
                        jc = colb + j
                        slots_t[jc % P, jc // P] = slot_s[s0 : s0 + n]
        deg_loc = np.ones(nblk * P, np.float32)
        deg_loc[:npc] = deg[c * npc : (c + 1) * npc]
        deg_t = deg_loc.reshape(nblk, P).T.copy()  # [P, nblk]
        per_core.append(
            dict(
                idx16=np.tile(idx16, (8, 1)),  # [128, tot16]
                slots=slots_t,
                degt=deg_t,
                degrow=deg_loc.reshape(1, -1).copy(),
            )
        )
    return plan, per_core


def _build_nc(plan, f0, f1, f2):
    """Build the SPMD bass program (same for every core)."""
    n_nodes, npc, nblk, hb = plan.n_nodes, plan.npc, plan.nblk, plan.hb
    rows = (hb, n_nodes - hb)  # rows of each half table
    nc = bacc.Bacc(
        "TRN2", target_bir_lowering=False, debug=False, num_devices=plan.n_cores
    )

    # I/O
    xT_d = nc.dram_tensor("xT", [f0, npc], F32, kind="ExternalInput")
    w0_d = nc.dram_tensor("W0", [f0, f1], F32, kind="ExternalInput")
    w1_d = nc.dram_tensor("W1", [f1, f2], F32, kind="ExternalInput")
    b0_d = nc.dram_tensor("b0", [1, f1], F32, kind="ExternalInput")
    b1_d = nc.dram_tensor("b1", [1, f2], F32, kind="ExternalInput")
    iota_d = nc.dram_tensor("iota", [P, P], F32, kind="ExternalInput")
    ident_d = nc.dram_tensor("ident", [P, P], F32, kind="ExternalInput")
    degt_d = nc.dram_tensor("degt", [P, nblk], F32, kind="ExternalInput")
    degrow_d = nc.dram_tensor("degrow", [1, nblk * P], F32, kind="ExternalInput")
    idx_d = nc.dram_tensor("idx16", [P, plan.tot16], I16, kind="ExternalInput")
    widx_d = nc.dram_tensor("widx", [P, 8], I16, kind="ExternalInput")
    slots_d = nc.dram_tensor("slots", [P, plan.ncols], F32, kind="ExternalInput")
    y_d = nc.dram_tensor("y", [npc, f2], F32, kind="ExternalOutput")

    rg = [list(range(plan.n_cores))]
    AF = mybir.ActivationFunctionType

    with tile.TileContext(nc) as tc:
        with (
            tc.tile_pool(name="dram", bufs=1, space="DRAM") as dramp,
            tc.tile_pool(name="const", bufs=1) as constp,
            tc.tile_pool(name="gath", bufs=4) as gpool,
            tc.tile_pool(name="sel", bufs=4) as spool,
            tc.tile_pool(name="stage", bufs=4) as stpool,
            tc.tile_pool(name="pgemm", bufs=2, space="PSUM") as pgemm,
            tc.tile_pool(name="pscat", bufs=2, space="PSUM") as pscat,
            tc.tile_pool(name="ptrans", bufs=2, space="PSUM") as ptrans,
        ):
            h1_loc = dramp.tile([npc, f1], BF16, name="h1_loc")
            h1_full = dramp.tile(
                [n_nodes, f1], BF16, addr_space="Shared", name="h1_full"
            )
            h2_loc = dramp.tile([npc, f2], F32, name="h2_loc")
            h2_full = dramp.tile(
                [n_nodes, f2], F32, addr_space="Shared", name="h2_full"
            )

            # ---- constants / metadata ----
            def load_const(name, dram, shape, dtype=F32):
                t = constp.tile(shape, dtype, name=name)
                nc.sync.dma_start(out=t[:], in_=dram[:])
                return t

            # ordered so the L0 GEMM -> AllGather chain starts ASAP; the big
            # gather metadata loads overlap with it
            xT_t = load_const("xT_t", xT_d, [f0, npc])
            w0_t = load_const("w0_t", w0_d, [f0, f1])
            degt_t = load_const("degt_t", degt_d, [P, nblk])
            w1_t = load_const("w1_t", w1_d, [f1, f2])
            b0_t = load_const("b0_t", b0_d, [1, f1])
            b1_t = load_const("b1_t", b1_d, [1, f2])
            iota_t = load_const("iota_t", iota_d, [P, P])
            ident_t = load_const("ident_t", ident_d, [P, P])
            degrow_t = load_const("degrow_t", degrow_d, [1, nblk * P])
            widx_t = load_const("widx_t", widx_d, [P, 8], I16)
            idx_t = load_const("idx_t", idx_d, [P, plan.tot16], I16)
            slots_t = load_const("slots_t", slots_d, [P, plan.ncols])

            # dinv = 1/sqrt(deg); sqdeg rows (flat, partition 0) for bias matmuls
            sq_t = constp.tile([P, nblk], F32, name="sq_t")
            nc.scalar.activation(sq_t[:], degt_t[:], AF.Sqrt)
            dinv_t = constp.tile([P, nblk], F32, name="dinv_t")
            nc.vector.reciprocal(dinv_t[:], sq_t[:])
            sqrow_t = constp.tile([1, nblk * P], F32, name="sqrow_t")
            nc.scalar.activation(sqrow_t[:], degrow_t[:], AF.Sqrt)

            x1T_t = constp.tile([f1, nblk * P], F32, name="x1T_t")

            # warm the Q7 dma_gather ucode (first gather pays ~29us icache
            # fill; do it under the GEMM+AllGather head instead)
            warm_t = constp.tile([P, 1, 64], F32, name="warm_t")
            nc.gpsimd.dma_gather(
                warm_t[:],
                ident_d[:, 0:64],
                widx_t[:, 0:8],
                128,
                128,
                64,
                elem_step=P,
                single_packet=False,
            )

            def gemm_layer(src_sbuf, w_t, fout, dst_dram, hdt):
                """dst_dram[rows] = dinv * (x @ W) for the local node rows."""
                for t in range(nblk):
                    wt = min(P, npc - t * P)
                    hp = pgemm.tile([P, fout], F32, name="hp")
                    nc.tensor.matmul(
                        hp[:wt, :],
                        src_sbuf[:, t * P : t * P + wt],
                        w_t[:],
                        start=True,
                        stop=True,
                    )
                    hs = stpool.tile([P, fout], hdt, name="hs")
                    nc.scalar.activation(
                        hs[:wt, :],
                        hp[:wt, :],
                        AF.Copy,
                        scale=dinv_t[:wt, t : t + 1],
                    )
                    nc.sync.dma_start(
                        out=dst_dram[t * P : t * P + wt, :], in_=hs[:wt, :]
                    )

            def scatter_layer(h_full, fout, bias_t, is_last, hdt):
                """For every dst block: gather + one-hot matmul scatter-add.

                Segments are SEG_ALIGN(64)-aligned inside each gather, so a
                128-edge chunk column can hold the tail of one block and the
                head of the next; those columns get two matmuls over partition
                ranges [0:64) / [64:128)."""
                g_of = plan.g_of
                for gi, blocks in enumerate(plan.groups):
                    gt = {}
                    for h in (0, 1):
                        nid = plan.gather_nid[(gi, h)]
                        if nid == 0:
                            continue
                        ncol = (nid + P - 1) // P
                        g_tile = gpool.tile(
                            [P, ncol, fout], hdt, tag="gath", name=f"g{gi}_{h}"
                        )
                        i0 = plan.seg_idx16[(gi, h)]
                        nc.gpsimd.dma_gather(
                            g_tile[:],
                            h_full[h * hb : h * hb + rows[h], :],
                            idx_t[:, i0 : i0 + nid // 16],
                            nid,
                            nid,
                            fout,
                            single_packet=False,
                        )
                        gt[h] = g_tile
                    for b in blocks:
                        wb = min(P, npc - b * P)
                        pb = pscat.tile([P, fout], F32, name="pb")
                        nc.tensor.matmul(
                            pb[:],
                            sqrow_t[0:1, b * P : (b + 1) * P],
                            bias_t[:],
                            start=True,
                            stop=False,
                        )
                        pieces = []  # (h, col, p0, p1)
                        sels = {}
                        spans = {}  # h -> (first_col, ncols)
                        for h in (0, 1):
                            sz = int(plan.SZ[b, h])
                            if sz == 0:
                                continue
                            off = plan.seg_off[(b, h)]
                            c_lo = off // P
                            c_hi = (off + sz - 1) // P
                            spans[h] = (c_lo, c_hi - c_lo + 1)
                            for c in range(c_lo, c_hi + 1):
                                p0 = max(0, off - P * c)
                                p1 = min(P, off + sz - P * c)
                                pieces.append((h, c, p0, p1))
                        for h, (c_lo, nch) in spans.items():
                            colb = plan.seg_colbase[(g_of[b], h)]
                            sel = spool.tile(
                                [P, nch, P], hdt, tag="sel", name="sel"
                            )
                            nc.vector.tensor_tensor(
                                out=sel[:],
                                in0=slots_t[
                                    :, colb + c_lo : colb + c_lo + nch
                                ].to_broadcast([P, nch, P]),
                                in1=iota_t[:, :]
                                .rearrange("p (a b) -> p a b", a=1)
                                .to_broadcast([P, nch, P]),
                                op=mybir.AluOpType.is_equal,
                            )
                            sels[h] = (sel, c_lo)
                        for k, (h, c, p0, p1) in enumerate(pieces):
                            sel, c_lo = sels[h]
                            nc.tensor.matmul(
                                pb[:],
                                sel[p0:p1, c - c_lo, :],
                                gt[h][p0:p1, c, :],
                                start=False,
                                stop=(k == len(pieces) - 1),
                            )
                        ob = stpool.tile([P, fout], F32, tag="ob", name="ob")
                        if is_last:
                            nc.scalar.activation(
                                ob[:wb, :],
                                pb[:wb, :],
                                AF.Copy,
                                scale=dinv_t[:wb, b : b + 1],
                            )
                            nc.sync.dma_start(
                                out=y_d[b * P : b * P + wb, :], in_=ob[:wb, :]
                            )
                        else:
                            nc.scalar.activation(
                                ob[:],
                                pb[:],
                                AF.Sigmoid,
                                scale=dinv_t[:, b : b + 1],
                            )
                            pt = ptrans.tile([P, P], F32, name="pt")
                            nc.tensor.transpose(pt[:], ob[:], ident_t[:])
                            nc.vector.tensor_copy(
                                x1T_t[:, b * P : (b + 1) * P], pt[:]
                            )

            # ---- layer 0 ----
            gemm_layer(xT_t, w0_t, f1, h1_loc, BF16)
            nc.gpsimd.collective_compute(
                "AllGather",
                mybir.AluOpType.bypass,
                replica_groups=rg,
                ins=[h1_loc[:, :].opt()],
                outs=[h1_full[:, :].opt()],
            )
            scatter_layer(h1_full, f1, b0_t, is_last=False, hdt=BF16)

            # ---- layer 1 ----
            gemm_layer(x1T_t, w1_t, f2, h2_loc, F32)
            nc.gpsimd.collective_compute(
                "AllGather",
                mybir.AluOpType.bypass,
                replica_groups=rg,
                ins=[h2_loc[:, :].opt()],
                outs=[h2_full[:, :].opt()],
            )
            scatter_layer(h2_full, f2, b1_t, is_last=True, hdt=F32)

    nc.compile()
    return nc


def _make_in_maps(x, W0, b0, W1, b1, plan, per_core):
    npc = plan.npc
    x = np.asarray(x, dtype=np.float32)
    shared = dict(
        W0=np.asarray(W0, np.float32).reshape(W0.shape[0], -1),
        W1=np.asarray(W1, np.float32).reshape(W1.shape[0], -1),
        b0=np.asarray(b0, np.float32).reshape(1, -1),
        b1=np.asarray(b1, np.float32).reshape(1, -1),
        iota=np.tile(np.arange(P, dtype=np.float32)[None, :], (P, 1)).copy(),
        ident=np.eye(P, dtype=np.float32),
    )
    in_maps = []
    for c in range(plan.n_cores):
        m = dict(shared)
        m["xT"] = np.ascontiguousarray(x[c * npc : (c + 1) * npc, :].T)
        m["idx16"] = per_core[c]["idx16"]
        m["widx"] = np.zeros((P, 8), np.int16)
        m["slots"] = per_core[c]["slots"]
        m["degt"] = per_core[c]["degt"]
        m["degrow"] = per_core[c]["degrow"]
        in_maps.append(m)
    return in_maps


_CACHE = {}


def build(x, edges, W0, b0, W1, b1, n_nodes=N_NODES, n_cores=N_CORES,
          gb=GROUP_BLOCKS):
    """Returns (nc, in_maps, plan). Cached on the edge structure size."""
    plan, per_core = _build_metadata(edges, n_nodes, n_cores, gb)
    key = (n_nodes, n_cores, gb, tuple(plan.SZ.reshape(-1).tolist()))
    if key not in _CACHE:
        _CACHE[key] = _build_nc(plan, x.shape[1], W0.shape[1], W1.shape[1])
    nc = _CACHE[key]
    in_maps = _make_in_maps(x, W0, b0, W1, b1, plan, per_core)
    return nc, in_maps, plan


def kernel(x, edges, W0, b0, W1, b1):
    x = np.asarray(x)
    nc, in_maps, plan = build(x, edges, W0, b0, W1, b1)
    res = run_bass_kernel_spmd(nc, in_maps, list(range(plan.n_cores)))
    y = np.concatenate([r["y"] for r in res.results], axis=0)
    return y.astype(np.float32)



# revision 5
# speedup vs baseline: 1.6682x; 1.6682x over previous
"""Bass/Trainium2 SPMD kernel for a 2-layer GCN encoder.

Math (per reference):
    src/dst = edges + self-loops
    deg[v]  = #edges with dst==v (incl self-loop);  dinv = 1/sqrt(deg)
    layer(x, W, b): out[d] = dinv[d] * sum_{e: dst_e==d} dinv[src_e] * (x@W)[src_e] + b
    y = layer1(sigmoid(layer0(x, W0, b0)), W1, b1)

Distribution: nodes are sharded contiguously across 8 cores (6250 each).
Edges are owned by the destination core.  Each core:
  1. GEMM on its x rows, pre-scales rows by dinv (so the per-edge weight
     dinv[src]*dinv[dst] factorizes into a row pre-scale and an output
     post-scale), AllGathers the scaled features.
  2. For each 128-row destination block, gathers the source rows of its
     edges (dma_gather, int16 indices => the node table is split in two
     halves), builds one-hot scatter matrices on the vector engine
     (iota == slot), and scatter-adds via TensorE matmuls accumulating in
     PSUM.  Bias enters as a rank-1 matmul (sqrt(deg) x b), so the final
     PSUM->SBUF copy can apply the dinv post-scale (and sigmoid) in one
     ScalarE activation.
"""

import math

import numpy as np

import concourse.bacc as bacc
import concourse.bass as bass
import concourse.mybir as mybir
import concourse.tile as tile
from concourse.bass_utils import run_bass_kernel_spmd

P = 128
F32 = mybir.dt.float32
BF16 = mybir.dt.bfloat16
I16 = mybir.dt.int16

# Full-problem constants
N_NODES = 50000
N_CORES = 8
F0, F1, F2 = 128, 128, 64
GROUP_BLOCKS = 3  # dst blocks per dma_gather batch
# Per-(block,half) edge-segment alignment. Must stay 128: sub-128 matmul
# pieces with different base partitions back-to-back hard-crash the PE
# (verified on HW: K64@p0 directly followed by K64@p64 aborts the NEFF).
SEG_ALIGN = 128


def _round_up(x, m):
    return (x + m - 1) // m * m


class Plan:
    """Compile-time schedule, identical across cores (SPMD)."""

    def __init__(self, n_nodes, n_cores, gb):
        assert n_nodes % n_cores == 0
        self.n_nodes = n_nodes
        self.n_cores = n_cores
        self.npc = n_nodes // n_cores
        self.nblk = math.ceil(self.npc / P)
        self.hb = (n_nodes + 1) // 2  # half boundary for int16 gather indices
        assert self.hb <= 32768
        self.gb = gb
        self.groups = [
            list(range(i, min(i + gb, self.nblk))) for i in range(0, self.nblk, gb)
        ]
        self.g_of = {}
        for gi, blocks in enumerate(self.groups):
            for b in blocks:
                self.g_of[b] = gi
        # filled by finalize(): per-(blk, half) uniform padded sizes
        self.SZ = None  # [nblk, 2] int, multiples of SEG_ALIGN
        self.seg_off = {}  # (b, h) -> edge offset within its gather
        self.seg_idx16 = {}  # (g_idx, h) -> int16-column base of that gather
        self.seg_colbase = {}  # (g_idx, h) -> global chunk-column base
        self.gather_nid = {}  # (g_idx, h) -> num idxs
        self.ncols = 0
        self.tot16 = 0

    def finalize(self, sz):
        self.SZ = sz
        col = 0
        i16 = 0
        for gi, blocks in enumerate(self.groups):
            for h in (0, 1):
                off = 0
                for b in blocks:
                    self.seg_off[(b, h)] = off
                    off += int(self.SZ[b, h])
                nid = off
                self.gather_nid[(gi, h)] = nid
                self.seg_idx16[(gi, h)] = i16
                self.seg_colbase[(gi, h)] = col
                col += (nid + P - 1) // P
                i16 += nid // 16
        self.ncols = col
        self.tot16 = i16


def _build_metadata(edges, n_nodes, n_cores, gb=GROUP_BLOCKS):
    """Host-side integer preprocessing: shard + sort edges, build gather
    indices / slot vectors / degree tables.  Returns (plan, per_core dict)."""
    plan = Plan(n_nodes, n_cores, gb)
    npc, nblk, hb = plan.npc, plan.nblk, plan.hb

    loop = np.arange(n_nodes, dtype=np.int64)
    src = np.concatenate([np.asarray(edges[0], dtype=np.int64), loop])
    dst = np.concatenate([np.asarray(edges[1], dtype=np.int64), loop])
    deg = np.bincount(dst, minlength=n_nodes).astype(np.float32)

    owner = dst // npc
    ldst = dst % npc
    blk = ldst // P
    slot = (ldst % P).astype(np.float32)
    half = (src >= hb).astype(np.int64)
    cell = ((owner * nblk) + blk) * 2 + half
    order = np.lexsort((src, cell))
    cell_s = cell[order]
    src_s = src[order]
    slot_s = slot[order]

    ncells = n_cores * nblk * 2
    counts = np.bincount(cell_s, minlength=ncells).reshape(n_cores, nblk, 2)
    starts = np.concatenate([[0], np.cumsum(counts.reshape(-1))])[:-1].reshape(
        n_cores, nblk, 2
    )
    sz = np.maximum(counts.max(axis=0), 0)
    sz = (np.ceil(sz / SEG_ALIGN).astype(np.int64)) * SEG_ALIGN  # [nblk, 2]
    plan.finalize(sz)

    ncols = plan.ncols
    tot16 = plan.tot16

    per_core = []
    for c in range(n_cores):
        idx16 = np.zeros((16, tot16), np.int16)
        slots_t = np.full((P, ncols), -1.0, np.float32)
        for gi, blocks in enumerate(plan.groups):
            for h in (0, 1):
                i16b = plan.seg_idx16[(gi, h)] * 16
                colb = plan.seg_colbase[(gi, h)] * P
                for b in blocks:
                    n = int(counts[c, b, h])
                    s0 = int(starts[c, b, h])
                    if n:
                        j = plan.seg_off[(b, h)] + np.arange(n)
                        seg_src = (src_s[s0 : s0 + n] - h * hb).astype(np.int16)
                        ji = i16b + j
                        idx16[ji % 16, ji // 16] = seg_src
                        jc = colb + j
                        slots_t[jc % P, jc // P] = slot_s[s0 : s0 + n]
        deg_loc = np.ones(nblk * P, np.float32)
        deg_loc[:npc] = deg[c * npc : (c + 1) * npc]
        deg_t = deg_loc.reshape(nblk, P).T.copy()  # [P, nblk]
        per_core.append(
            dict(
                idx16=np.tile(idx16, (8, 1)),  # [128, tot16]
                slots=slots_t,
                degt=deg_t,
                degrow=deg_loc.reshape(1, -1).copy(),
            )
        )
    return plan, per_core


def _build_nc(plan, f0, f1, f2):
    """Build the SPMD bass program (same for every core)."""
    n_nodes, npc, nblk, hb = plan.n_nodes, plan.npc, plan.nblk, plan.hb
    rows = (hb, n_nodes - hb)  # rows of each half table
    nc = bacc.Bacc(
        "TRN2",
        target_bir_lowering=False,
        debug=False,
        num_devices=plan.n_cores,
        num_swdge_queues=4,
    )

    # I/O
    xT_d = nc.dram_tensor("xT", [f0, npc], F32, kind="ExternalInput")
    w0_d = nc.dram_tensor("W0", [f0, f1], F32, kind="ExternalInput")
    w1_d = nc.dram_tensor("W1", [f1, f2], F32, kind="ExternalInput")
    b0_d = nc.dram_tensor("b0", [1, f1], F32, kind="ExternalInput")
    b1_d = nc.dram_tensor("b1", [1, f2], F32, kind="ExternalInput")
    iota_d = nc.dram_tensor("iota", [P, P], F32, kind="ExternalInput")
    ident_d = nc.dram_tensor("ident", [P, P], F32, kind="ExternalInput")
    degt_d = nc.dram_tensor("degt", [P, nblk], F32, kind="ExternalInput")
    degrow_d = nc.dram_tensor("degrow", [1, nblk * P], F32, kind="ExternalInput")
    idx_d = nc.dram_tensor("idx16", [P, plan.tot16], I16, kind="ExternalInput")
    widx_d = nc.dram_tensor("widx", [P, 8], I16, kind="ExternalInput")
    slots_d = nc.dram_tensor("slots", [P, plan.ncols], F32, kind="ExternalInput")
    y_d = nc.dram_tensor("y", [npc, f2], F32, kind="ExternalOutput")

    rg = [list(range(plan.n_cores))]
    AF = mybir.ActivationFunctionType

    with tile.TileContext(nc) as tc:
        with (
            tc.tile_pool(name="dram", bufs=1, space="DRAM") as dramp,
            tc.tile_pool(name="const", bufs=1) as constp,
            tc.tile_pool(name="gath", bufs=6) as gpool,
            tc.tile_pool(name="sel", bufs=4) as spool,
            tc.tile_pool(name="stage", bufs=4) as stpool,
            tc.tile_pool(name="pgemm", bufs=2, space="PSUM") as pgemm,
            tc.tile_pool(name="pscat", bufs=2, space="PSUM") as pscat,
            tc.tile_pool(name="ptrans", bufs=2, space="PSUM") as ptrans,
        ):
            h1_loc = dramp.tile([npc, f1], BF16, name="h1_loc")
            h1_full = dramp.tile(
                [n_nodes, f1], BF16, addr_space="Shared", name="h1_full"
            )
            h2_loc = dramp.tile([npc, f2], F32, name="h2_loc")
            h2_full = dramp.tile(
                [n_nodes, f2], F32, addr_space="Shared", name="h2_full"
            )

            # ---- constants / metadata ----
            def load_const(name, dram, shape, dtype=F32):
                t = constp.tile(shape, dtype, name=name)
                nc.sync.dma_start(out=t[:], in_=dram[:])
                return t

            # ordered so the L0 GEMM -> AllGather chain starts ASAP; the big
            # gather metadata loads overlap with it
            xT_t = load_const("xT_t", xT_d, [f0, npc])
            w0_t = load_const("w0_t", w0_d, [f0, f1])
            degt_t = load_const("degt_t", degt_d, [P, nblk])
            w1_t = load_const("w1_t", w1_d, [f1, f2])
            b0_t = load_const("b0_t", b0_d, [1, f1])
            b1_t = load_const("b1_t", b1_d, [1, f2])
            iota_t = load_const("iota_t", iota_d, [P, P])
            ident_t = load_const("ident_t", ident_d, [P, P])
            degrow_t = load_const("degrow_t", degrow_d, [1, nblk * P])
            widx_t = load_const("widx_t", widx_d, [P, 8], I16)
            idx_t = load_const("idx_t", idx_d, [P, plan.tot16], I16)
            slots_t = load_const("slots_t", slots_d, [P, plan.ncols])

            # dinv = 1/sqrt(deg); sqdeg rows (flat, partition 0) for bias matmuls
            sq_t = constp.tile([P, nblk], F32, name="sq_t")
            nc.scalar.activation(sq_t[:], degt_t[:], AF.Sqrt)
            dinv_t = constp.tile([P, nblk], F32, name="dinv_t")
            nc.vector.reciprocal(dinv_t[:], sq_t[:])
            sqrow_t = constp.tile([1, nblk * P], F32, name="sqrow_t")
            nc.scalar.activation(sqrow_t[:], degrow_t[:], AF.Sqrt)

            x1T_t = constp.tile([f1, nblk * P], F32, name="x1T_t")

            # warm the Q7 dma_gather ucode (first gather pays ~29us icache
            # fill; do it under the GEMM+AllGather head instead).  Warm every
            # SWDGE queue so the first real gather on each skips ring init.
            for qn in range(4):
                warm_t = constp.tile([P, 1, 64], F32, name=f"warm_t{qn}")
                nc.gpsimd.dma_gather(
                    warm_t[:],
                    ident_d[:, 0:64],
                    widx_t[:, 0:8],
                    128,
                    128,
                    64,
                    elem_step=P,
                    single_packet=False,
                    queue_num=qn,
                )

            def gemm_layer(src_sbuf, w_t, fout, dst_dram, hdt):
                """dst_dram[rows] = dinv * (x @ W) for the local node rows."""
                for t in range(nblk):
                    wt = min(P, npc - t * P)
                    hp = pgemm.tile([P, fout], F32, name="hp")
                    nc.tensor.matmul(
                        hp[:wt, :],
                        src_sbuf[:, t * P : t * P + wt],
                        w_t[:],
                        start=True,
                        stop=True,
                    )
                    hs = stpool.tile([P, fout], hdt, name="hs")
                    nc.scalar.activation(
                        hs[:wt, :],
                        hp[:wt, :],
                        AF.Copy,
                        scale=dinv_t[:wt, t : t + 1],
                    )
                    nc.sync.dma_start(
                        out=dst_dram[t * P : t * P + wt, :], in_=hs[:wt, :]
                    )

            def scatter_layer(h_full, fout, bias_t, is_last, hdt):
                """For every dst block: gather + one-hot matmul scatter-add.

                Segments are SEG_ALIGN(64)-aligned inside each gather, so a
                128-edge chunk column can hold the tail of one block and the
                head of the next; those columns get two matmuls over partition
                ranges [0:64) / [64:128)."""
                g_of = plan.g_of
                for gi, blocks in enumerate(plan.groups):
                    gt = {}
                    for h in (0, 1):
                        nid = plan.gather_nid[(gi, h)]
                        if nid == 0:
                            continue
                        ncol = (nid + P - 1) // P
                        g_tile = gpool.tile(
                            [P, ncol, fout], hdt, tag="gath", name=f"g{gi}_{h}"
                        )
                        i0 = plan.seg_idx16[(gi, h)]
                        nc.gpsimd.dma_gather(
                            g_tile[:],
                            h_full[h * hb : h * hb + rows[h], :],
                            idx_t[:, i0 : i0 + nid // 16],
                            nid,
                            nid,
                            fout,
                            single_packet=False,
                            queue_num=(2 * gi + h) % 4,
                        )
                        gt[h] = g_tile
                    for b in blocks:
                        wb = min(P, npc - b * P)
                        pb = pscat.tile([P, fout], F32, name="pb")
                        nc.tensor.matmul(
                            pb[:],
                            sqrow_t[0:1, b * P : (b + 1) * P],
                            bias_t[:],
                            start=True,
                            stop=False,
                        )
                        pieces = []  # (h, col, p0, p1)
                        sels = {}
                        spans = {}  # h -> (first_col, ncols)
                        for h in (0, 1):
                            sz = int(plan.SZ[b, h])
                            if sz == 0:
                                continue
                            off = plan.seg_off[(b, h)]
                            c_lo = off // P
                            c_hi = (off + sz - 1) // P
                            spans[h] = (c_lo, c_hi - c_lo + 1)
                            for c in range(c_lo, c_hi + 1):
                                p0 = max(0, off - P * c)
                                p1 = min(P, off + sz - P * c)
                                pieces.append((h, c, p0, p1))
                        for h, (c_lo, nch) in spans.items():
                            colb = plan.seg_colbase[(g_of[b], h)]
                            sel = spool.tile(
                                [P, nch, P], hdt, tag="sel", name="sel"
                            )
                            nc.vector.tensor_tensor(
                                out=sel[:],
                                in0=slots_t[
                                    :, colb + c_lo : colb + c_lo + nch
                                ].to_broadcast([P, nch, P]),
                                in1=iota_t[:, :]
                                .rearrange("p (a b) -> p a b", a=1)
                                .to_broadcast([P, nch, P]),
                                op=mybir.AluOpType.is_equal,
                            )
                            sels[h] = (sel, c_lo)
                        for k, (h, c, p0, p1) in enumerate(pieces):
                            sel, c_lo = sels[h]
                            nc.tensor.matmul(
                                pb[:],
                                sel[p0:p1, c - c_lo, :],
                                gt[h][p0:p1, c, :],
                                start=False,
                                stop=(k == len(pieces) - 1),
                            )
                        ob = stpool.tile([P, fout], F32, tag="ob", name="ob")
                        if is_last:
                            nc.scalar.activation(
                                ob[:wb, :],
                                pb[:wb, :],
                                AF.Copy,
                                scale=dinv_t[:wb, b : b + 1],
                            )
                            nc.sync.dma_start(
                                out=y_d[b * P : b * P + wb, :], in_=ob[:wb, :]
                            )
                        else:
                            nc.scalar.activation(
                                ob[:],
                                pb[:],
                                AF.Sigmoid,
                                scale=dinv_t[:, b : b + 1],
                            )
                            pt = ptrans.tile([P, P], F32, name="pt")
                            nc.tensor.transpose(pt[:], ob[:], ident_t[:])
                            nc.vector.tensor_copy(
                                x1T_t[:, b * P : (b + 1) * P], pt[:]
                            )

            # ---- layer 0 ----
            gemm_layer(xT_t, w0_t, f1, h1_loc, BF16)
            nc.gpsimd.collective_compute(
                "AllGather",
                mybir.AluOpType.bypass,
                replica_groups=rg,
                ins=[h1_loc[:, :].opt()],
                outs=[h1_full[:, :].opt()],
            )
            scatter_layer(h1_full, f1, b0_t, is_last=False, hdt=BF16)

            # ---- layer 1 ----
            gemm_layer(x1T_t, w1_t, f2, h2_loc, F32)
            nc.gpsimd.collective_compute(
                "AllGather",
                mybir.AluOpType.bypass,
                replica_groups=rg,
                ins=[h2_loc[:, :].opt()],
                outs=[h2_full[:, :].opt()],
            )
            scatter_layer(h2_full, f2, b1_t, is_last=True, hdt=F32)

    nc.compile()
    return nc


def _make_in_maps(x, W0, b0, W1, b1, plan, per_core):
    npc = plan.npc
    x = np.asarray(x, dtype=np.float32)
    shared = dict(
        W0=np.asarray(W0, np.float32).reshape(W0.shape[0], -1),
        W1=np.asarray(W1, np.float32).reshape(W1.shape[0], -1),
        b0=np.asarray(b0, np.float32).reshape(1, -1),
        b1=np.asarray(b1, np.float32).reshape(1, -1),
        iota=np.tile(np.arange(P, dtype=np.float32)[None, :], (P, 1)).copy(),
        ident=np.eye(P, dtype=np.float32),
    )
    in_maps = []
    for c in range(plan.n_cores):
        m = dict(shared)
        m["xT"] = np.ascontiguousarray(x[c * npc : (c + 1) * npc, :].T)
        m["idx16"] = per_core[c]["idx16"]
        m["widx"] = np.zeros((P, 8), np.int16)
        m["slots"] = per_core[c]["slots"]
        m["degt"] = per_core[c]["degt"]
        m["degrow"] = per_core[c]["degrow"]
        in_maps.append(m)
    return in_maps


_CACHE = {}


def build(x, edges, W0, b0, W1, b1, n_nodes=N_NODES, n_cores=N_CORES,
          gb=GROUP_BLOCKS):
    """Returns (nc, in_maps, plan). Cached on the edge structure size."""
    plan, per_core = _build_metadata(edges, n_nodes, n_cores, gb)
    key = (n_nodes, n_cores, gb, tuple(plan.SZ.reshape(-1).tolist()))
    if key not in _CACHE:
        _CACHE[key] = _build_nc(plan, x.shape[1], W0.shape[1], W1.shape[1])
    nc = _CACHE[key]
    in_maps = _make_in_maps(x, W0, b0, W1, b1, plan, per_core)
    return nc, in_maps, plan


def kernel(x, edges, W0, b0, W1, b1):
    x = np.asarray(x)
    nc, in_maps, plan = build(x, edges, W0, b0, W1, b1)
    res = run_bass_kernel_spmd(nc, in_maps, list(range(plan.n_cores)))
    y = np.concatenate([r["y"] for r in res.results], axis=0)
    return y.astype(np.float32)



# revision 9
# speedup vs baseline: 2.2419x; 1.3439x over previous
"""Bass/Trainium2 SPMD kernel for a 2-layer GCN encoder.

Math (per reference):
    src/dst = edges + self-loops
    deg[v]  = #edges with dst==v (incl self-loop);  dinv = 1/sqrt(deg)
    layer(x, W, b): out[d] = dinv[d] * sum_{e: dst_e==d} dinv[src_e] * (x@W)[src_e] + b
    y = layer1(sigmoid(layer0(x, W0, b0)), W1, b1)

Distribution: nodes are sharded contiguously across 8 cores (6250 each).
Edges are owned by the destination core.  Each core:
  1. GEMM (bf16) on its x rows, pre-scales rows by dinv (so the per-edge
     weight dinv[src]*dinv[dst] factorizes into a row pre-scale and an
     output post-scale), AllGathers the scaled features.
  2. For each 128-row destination block, gathers the source rows of its
     non-self-loop edges (dma_gather striped over all 4 SWDGE queues,
     int16 indices => the node table is split in two halves), builds
     one-hot scatter matrices on the vector engine (iota == slot), and
     scatter-adds via TensorE matmuls accumulating in PSUM.  Self-loop
     plus bias contributions enter via one identity matmul per block
     whose rhs is (local stored rows + host-precomputed sqrt(deg)*b),
     so the final PSUM->SBUF copy applies the dinv post-scale (and
     sigmoid) in one ScalarE activation.
Everything on the PE runs in bf16 (layer-1's 64-wide f32 rows are kept
as 128-wide zero-padded bf16 rows so dma_gather's 256B-row constraint
holds and the matmuls avoid fp32 LOW_HIGH double passes).
"""

import math

import numpy as np

import concourse.bacc as bacc
import concourse.bass as bass
import concourse.mybir as mybir
import concourse.tile as tile
from concourse.bass_utils import run_bass_kernel_spmd

P = 128
F32 = mybir.dt.float32
BF16 = mybir.dt.bfloat16
I16 = mybir.dt.int16

# Full-problem constants
N_NODES = 50000
N_CORES = 8
F0, F1, F2 = 128, 128, 64
GROUP_BLOCKS = 3  # dst blocks per dma_gather batch
# Per-(block,half) edge-segment alignment. Must stay 128: sub-128 matmul
# pieces with different base partitions back-to-back hard-crash the PE
# (verified on HW: K64@p0 directly followed by K64@p64 aborts the NEFF).
SEG_ALIGN = 128


def _round_up(x, m):
    return (x + m - 1) // m * m


class Plan:
    """Compile-time schedule, identical across cores (SPMD)."""

    def __init__(self, n_nodes, n_cores, gb):
        assert n_nodes % n_cores == 0
        self.n_nodes = n_nodes
        self.n_cores = n_cores
        self.npc = n_nodes // n_cores
        self.nblk = math.ceil(self.npc / P)
        self.hb = (n_nodes + 1) // 2  # half boundary for int16 gather indices
        assert self.hb <= 32768
        self.gb = gb
        self.groups = [
            list(range(i, min(i + gb, self.nblk))) for i in range(0, self.nblk, gb)
        ]
        self.g_of = {}
        for gi, blocks in enumerate(self.groups):
            for b in blocks:
                self.g_of[b] = gi
        # filled by finalize(): per-(blk, half) uniform padded sizes
        self.SZ = None  # [nblk, 2] int, multiples of SEG_ALIGN
        self.seg_off = {}  # (b, h) -> edge offset within its gather
        self.seg_idx16 = {}  # (g_idx, h) -> int16-column base of that gather
        self.seg_colbase = {}  # (g_idx, h) -> global chunk-column base
        self.gather_nid = {}  # (g_idx, h) -> num idxs
        self.ncols = 0
        self.tot16 = 0

    def finalize(self, sz):
        self.SZ = sz
        col = 0
        i16 = 0
        for gi, blocks in enumerate(self.groups):
            for h in (0, 1):
                off = 0
                for b in blocks:
                    self.seg_off[(b, h)] = off
                    off += int(self.SZ[b, h])
                nid = off
                self.gather_nid[(gi, h)] = nid
                self.seg_idx16[(gi, h)] = i16
                self.seg_colbase[(gi, h)] = col
                col += (nid + P - 1) // P
                i16 += nid // 16
        self.ncols = col
        self.tot16 = i16


def _build_metadata(edges, n_nodes, n_cores, gb=GROUP_BLOCKS):
    """Host-side integer preprocessing: shard + sort edges, build gather
    indices / slot vectors / degree tables.  Returns (plan, per_core dict).

    Self-loops are NOT materialized as edges (they enter via an identity
    matmul on the locally-stored rows); they only contribute to deg."""
    plan = Plan(n_nodes, n_cores, gb)
    npc, nblk, hb = plan.npc, plan.nblk, plan.hb

    src = np.asarray(edges[0], dtype=np.int64)
    dst = np.asarray(edges[1], dtype=np.int64)
    deg = (np.bincount(dst, minlength=n_nodes) + 1).astype(np.float32)
    dinv = (1.0 / np.sqrt(deg)).astype(np.float32)
    sqdeg = np.sqrt(deg).astype(np.float32)

    owner = dst // npc
    ldst = dst % npc
    blk = ldst // P
    slot = (ldst % P).astype(np.float32)
    half = (src >= hb).astype(np.int64)
    cell = ((owner * nblk) + blk) * 2 + half
    order = np.lexsort((src, cell))
    cell_s = cell[order]
    src_s = src[order]
    slot_s = slot[order]

    ncells = n_cores * nblk * 2
    counts = np.bincount(cell_s, minlength=ncells).reshape(n_cores, nblk, 2)
    starts = np.concatenate([[0], np.cumsum(counts.reshape(-1))])[:-1].reshape(
        n_cores, nblk, 2
    )
    sz = np.maximum(counts.max(axis=0), 0)
    sz = (np.ceil(sz / SEG_ALIGN).astype(np.int64)) * SEG_ALIGN  # [nblk, 2]
    plan.finalize(sz)

    ncols = plan.ncols
    tot16 = plan.tot16

    per_core = []
    for c in range(n_cores):
        idx16 = np.zeros((16, tot16), np.int16)
        slots_t = np.full((P, ncols), -1.0, np.float32)
        for gi, blocks in enumerate(plan.groups):
            for h in (0, 1):
                i16b = plan.seg_idx16[(gi, h)] * 16
                colb = plan.seg_colbase[(gi, h)] * P
                for b in blocks:
                    n = int(counts[c, b, h])
                    s0 = int(starts[c, b, h])
                    if n:
                        j = plan.seg_off[(b, h)] + np.arange(n)
                        seg_src = (src_s[s0 : s0 + n] - h * hb).astype(np.int16)
                        ji = i16b + j
                        idx16[ji % 16, ji // 16] = seg_src
                        jc = colb + j
                        slots_t[jc % P, jc // P] = slot_s[s0 : s0 + n]
        deg_loc = np.ones(nblk * P, np.float32)
        deg_loc[:npc] = deg[c * npc : (c + 1) * npc]
        dinv_loc = np.ones(nblk * P, np.float32)
        dinv_loc[:npc] = dinv[c * npc : (c + 1) * npc]
        sq_loc = np.ones(nblk * P, np.float32)
        sq_loc[:npc] = sqdeg[c * npc : (c + 1) * npc]
        per_core.append(
            dict(
                idx16=np.tile(idx16, (8, 1)),  # [128, tot16]
                slots=slots_t.astype(np.float32),
                dinvt=dinv_loc.reshape(nblk, P).T.copy(),  # [P, nblk]
                sq=sq_loc.reshape(nblk, P).T.copy(),  # [P, nblk]
            )
        )
    return plan, per_core


def _build_nc(plan, f0, f1, f2):
    """Build the SPMD bass program (same for every core)."""
    n_nodes, npc, nblk, hb = plan.n_nodes, plan.npc, plan.nblk, plan.hb
    rows = (hb, n_nodes - hb)  # rows of each half table
    nc = bacc.Bacc(
        "TRN2",
        target_bir_lowering=False,
        debug=False,
        num_devices=plan.n_cores,
        num_swdge_queues=4,
    )

    # I/O (xT/W in bf16: the whole PE pipeline is bf16)
    xT_d = nc.dram_tensor("xT", [f0, npc], BF16, kind="ExternalInput")
    w0_d = nc.dram_tensor("W0", [f0, f1], BF16, kind="ExternalInput")
    w1_d = nc.dram_tensor("W1", [f1, f2], BF16, kind="ExternalInput")
    bias0_d = nc.dram_tensor("bias0", [P, nblk * f1], BF16, kind="ExternalInput")
    bias1_d = nc.dram_tensor("bias1", [P, nblk * f2], BF16, kind="ExternalInput")
    iota_d = nc.dram_tensor("iota", [P, P], BF16, kind="ExternalInput")
    ident_d = nc.dram_tensor("ident", [P, P], BF16, kind="ExternalInput")
    identf_d = nc.dram_tensor("identf", [P, P], F32, kind="ExternalInput")
    dinv_d = nc.dram_tensor("dinvt", [P, nblk], F32, kind="ExternalInput")
    idx_d = nc.dram_tensor("idx16", [P, plan.tot16], I16, kind="ExternalInput")
    widx_d = nc.dram_tensor("widx", [P, 8], I16, kind="ExternalInput")
    slots_d = nc.dram_tensor("slots", [P, plan.ncols], BF16, kind="ExternalInput")
    y_d = nc.dram_tensor("y", [npc, f2], F32, kind="ExternalOutput")

    rg = [list(range(plan.n_cores))]
    AF = mybir.ActivationFunctionType

    with tile.TileContext(nc) as tc:
        with (
            tc.tile_pool(name="dram", bufs=1, space="DRAM") as dramp,
            tc.tile_pool(name="const", bufs=1) as constp,
            tc.tile_pool(name="gath", bufs=6) as gpool,
            tc.tile_pool(name="sel", bufs=4) as spool,
            tc.tile_pool(name="stage", bufs=4) as stpool,
            tc.tile_pool(name="pgemm", bufs=2, space="PSUM") as pgemm,
            tc.tile_pool(name="pscat", bufs=2, space="PSUM") as pscat,
            tc.tile_pool(name="ptrans", bufs=2, space="PSUM") as ptrans,
        ):
            h1_loc = dramp.tile([npc, f1], BF16, name="h1_loc")
            h1_full = dramp.tile(
                [n_nodes, f1], BF16, addr_space="Shared", name="h1_full"
            )
            # layer-1 table rows are zero-padded to 128 bf16 so dma_gather's
            # 256B-row minimum holds and the scatter matmuls stay bf16
            h2_loc = dramp.tile([npc, P], BF16, name="h2_loc")
            h2_full = dramp.tile(
                [n_nodes, P], BF16, addr_space="Shared", name="h2_full"
            )

            # ---- constants / metadata ----
            def load_const(name, dram, shape, dtype=F32):
                t = constp.tile(shape, dtype, name=name)
                nc.sync.dma_start(out=t[:], in_=dram[:])
                return t

            # ordered so the L0 GEMM -> AllGather chain starts ASAP; the big
            # gather metadata loads overlap with it
            xT_t = load_const("xT_t", xT_d, [f0, npc], BF16)
            w0_t = load_const("w0_t", w0_d, [f0, f1], BF16)
            dinv_t = load_const("dinv_t", dinv_d, [P, nblk])
            w1_t = load_const("w1_t", w1_d, [f1, f2], BF16)
            iota_t = load_const("iota_t", iota_d, [P, P], BF16)
            ident_t = load_const("ident_t", ident_d, [P, P], BF16)
            identf_t = load_const("identf_t", identf_d, [P, P], F32)
            widx_t = load_const("widx_t", widx_d, [P, 8], I16)
            bias0_t = load_const("bias0_t", bias0_d, [P, nblk * f1], BF16)
            bias1_t = load_const("bias1_t", bias1_d, [P, nblk * f2], BF16)
            idx_t = load_const("idx_t", idx_d, [P, plan.tot16], I16)
            slots_t = load_const("slots_t", slots_d, [P, plan.ncols], BF16)

            # per-layer SBUF-resident stored rows (pre-scaled GEMM outputs)
            # and their bias-augmented copies for the identity matmuls
            h0s_t = constp.tile([P, nblk, f1], BF16, name="h0s_t")
            h1s_t = constp.tile([P, nblk, P], BF16, name="h1s_t")
            nc.vector.memset(h0s_t[:], 0.0)
            nc.vector.memset(h1s_t[:], 0.0)
            b0s_t = constp.tile([P, nblk, f1], BF16, name="b0s_t")
            b1s_t = constp.tile([P, nblk, f2], BF16, name="b1s_t")

            x1T_t = constp.tile([f1, nblk * P], BF16, name="x1T_t")

            # warm the Q7 dma_gather ucode (first gather pays ~29us icache
            # fill; do it under the GEMM+AllGather head instead).  Warm every
            # SWDGE queue so the first real gather on each skips ring init.
            for qn in range(4):
                warm_t = constp.tile([P, 1, 128], BF16, name=f"warm_t{qn}")
                nc.gpsimd.dma_gather(
                    warm_t[:],
                    ident_d[:, :],
                    widx_t[:, 0:8],
                    128,
                    128,
                    128,
                    elem_step=P,
                    single_packet=False,
                    queue_num=qn,
                )

            def gemm_layer(src_sbuf, w_t, fout, hs_t, hs_fout, dst_dram, bias_in,
                           bs_t):
                """hs/dst rows = dinv * (x @ W) for the local node rows; also
                fill bs = hs + host-precomputed sqrt(deg)*b for the identity
                matmuls."""
                for t in range(nblk):
                    wt = min(P, npc - t * P)
                    hp = pgemm.tile([P, fout], F32, name="hp")
                    nc.tensor.matmul(
                        hp[:wt, :],
                        src_sbuf[:, t * P : t * P + wt],
                        w_t[:],
                        start=True,
                        stop=True,
                    )
                    nc.scalar.activation(
                        hs_t[:wt, t, 0:fout],
                        hp[:wt, :],
                        AF.Copy,
                        scale=dinv_t[:wt, t : t + 1],
                    )
                    nc.sync.dma_start(
                        out=dst_dram[t * P : t * P + wt, :],
                        in_=hs_t[:wt, t, 0:hs_fout],
                    )
                    nc.vector.tensor_tensor(
                        out=bs_t[:, t, :],
                        in0=hs_t[:, t, 0:fout],
                        in1=bias_in[:, t * fout : (t + 1) * fout],
                        op=mybir.AluOpType.add,
                    )

            qctr = [0]

            def scatter_layer(h_full, fout, hs_t, bs_t, is_last):
                """For every dst block: gather + one-hot matmul scatter-add."""
                g_of = plan.g_of
                for gi, blocks in enumerate(plan.groups):
                    gt = {}
                    for h in (0, 1):
                        nid = plan.gather_nid[(gi, h)]
                        if nid == 0:
                            continue
                        ncol = (nid + P - 1) // P
                        g_tile = gpool.tile(
                            [P, ncol, P], BF16, tag="gath", name=f"g{gi}_{h}"
                        )
                        i0 = plan.seg_idx16[(gi, h)]
                        nc.gpsimd.dma_gather(
                            g_tile[:],
                            h_full[h * hb : h * hb + rows[h], :],
                            idx_t[:, i0 : i0 + nid // 16],
                            nid,
                            nid,
                            P,
                            single_packet=False,
                            queue_num=qctr[0] % 4,
                        )
                        qctr[0] += 1
                        gt[h] = g_tile
                    for b in blocks:
                        wb = min(P, npc - b * P)
                        pb = pscat.tile([P, fout], F32, name="pb")
                        # self-loop + bias: identity matmul on the local
                        # stored rows (+ sqrt(deg)*b, folded host-side)
                        nc.tensor.matmul(
                            pb[:],
                            ident_t[:],
                            bs_t[:, b, :],
                            start=True,
                            stop=False,
                        )
                        pieces = []  # (h, col, p0, p1)
                        sels = {}
                        spans = {}  # h -> (first_col, ncols)
                        for h in (0, 1):
                            sz = int(plan.SZ[b, h])
                            if sz == 0:
                                continue
                            off = plan.seg_off[(b, h)]
                            c_lo = off // P
                            c_hi = (off + sz - 1) // P
                            spans[h] = (c_lo, c_hi - c_lo + 1)
                            for c in range(c_lo, c_hi + 1):
                                p0 = max(0, off - P * c)
                                p1 = min(P, off + sz - P * c)
                                pieces.append((h, c, p0, p1))
                        for h, (c_lo, nch) in spans.items():
                            colb = plan.seg_colbase[(g_of[b], h)]
                            sel = spool.tile(
                                [P, nch, P], BF16, tag="sel", name="sel"
                            )
                            nc.vector.tensor_tensor(
                                out=sel[:],
                                in0=slots_t[
                                    :, colb + c_lo : colb + c_lo + nch
                                ].to_broadcast([P, nch, P]),
                                in1=iota_t[:, :]
                                .rearrange("p (a b) -> p a b", a=1)
                                .to_broadcast([P, nch, P]),
                                op=mybir.AluOpType.is_equal,
                            )
                            sels[h] = (sel, c_lo)
                        for k, (h, c, p0, p1) in enumerate(pieces):
                            sel, c_lo = sels[h]
                            nc.tensor.matmul(
                                pb[:],
                                sel[p0:p1, c - c_lo, :],
                                gt[h][p0:p1, c, 0:fout],
                                start=False,
                                stop=(k == len(pieces) - 1),
                            )
                        if is_last:
                            ob = stpool.tile([P, fout], F32, tag="ob", name="ob")
                            nc.scalar.activation(
                                ob[:wb, :],
                                pb[:wb, :],
                                AF.Copy,
                                scale=dinv_t[:wb, b : b + 1],
                            )
                            nc.sync.dma_start(
                                out=y_d[b * P : b * P + wb, :], in_=ob[:wb, :]
                            )
                        else:
                            ob = stpool.tile([P, fout], BF16, tag="ob", name="ob")
                            nc.scalar.activation(
                                ob[:],
                                pb[:],
                                AF.Sigmoid,
                                scale=dinv_t[:, b : b + 1],
                            )
                            pt = ptrans.tile([P, P], BF16, name="pt")
                            nc.tensor.transpose(pt[:], ob[:], ident_t[:])
                            nc.vector.tensor_copy(
                                x1T_t[:, b * P : (b + 1) * P], pt[:]
                            )

            # ---- layer 0 ----
            gemm_layer(xT_t, w0_t, f1, h0s_t, f1, h1_loc, bias0_t, b0s_t)
            nc.gpsimd.collective_compute(
                "AllGather",
                mybir.AluOpType.bypass,
                replica_groups=rg,
                ins=[h1_loc[:, :].opt()],
                outs=[h1_full[:, :].opt()],
            )
            scatter_layer(h1_full, f1, h0s_t, b0s_t, is_last=False)

            # ---- layer 1 ----
            gemm_layer(x1T_t, w1_t, f2, h1s_t, P, h2_loc, bias1_t, b1s_t)
            nc.gpsimd.collective_compute(
                "AllGather",
                mybir.AluOpType.bypass,
                replica_groups=rg,
                ins=[h2_loc[:, :].opt()],
                outs=[h2_full[:, :].opt()],
            )
            scatter_layer(h2_full, f2, h1s_t, b1s_t, is_last=True)

    nc.compile()
    return nc


def _make_in_maps(x, W0, b0, W1, b1, plan, per_core):
    npc, nblk = plan.npc, plan.nblk
    x = np.asarray(x, dtype=np.float32)
    shared = dict(
        W0=_bf16(np.asarray(W0, np.float32)),
        W1=_bf16(np.asarray(W1, np.float32)),
        iota=_bf16(np.tile(np.arange(P, dtype=np.float32)[None, :], (P, 1))),
        ident=_bf16(np.eye(P, dtype=np.float32)),
        identf=np.eye(P, dtype=np.float32),
    )
    b0v = np.asarray(b0, np.float32).reshape(-1)
    b1v = np.asarray(b1, np.float32).reshape(-1)
    in_maps = []
    for c in range(plan.n_cores):
        m = dict(shared)
        m["xT"] = _bf16(np.ascontiguousarray(x[c * npc : (c + 1) * npc, :].T))
        m["idx16"] = per_core[c]["idx16"]
        m["widx"] = np.zeros((P, 8), np.int16)
        m["slots"] = _bf16(per_core[c]["slots"])
        m["dinvt"] = per_core[c]["dinvt"]
        sq = per_core[c]["sq"]  # [P, nblk]
        m["bias0"] = _bf16(
            (sq[:, :, None] * b0v[None, None, :]).reshape(P, nblk * F1)
        )
        m["bias1"] = _bf16(
            (sq[:, :, None] * b1v[None, None, :]).reshape(P, nblk * F2)
        )
        in_maps.append(m)
    return in_maps


def _bf16(a):
    """float32 -> bfloat16 (round-to-nearest-even) as a uint16-viewed array
    that run_bass_kernel_spmd accepts for BF16 dram tensors."""
    import ml_dtypes

    return a.astype(ml_dtypes.bfloat16)


_CACHE = {}


def build(x, edges, W0, b0, W1, b1, n_nodes=N_NODES, n_cores=N_CORES,
          gb=GROUP_BLOCKS):
    """Returns (nc, in_maps, plan). Cached on the edge structure size."""
    plan, per_core = _build_metadata(edges, n_nodes, n_cores, gb)
    key = (n_nodes, n_cores, gb, tuple(plan.SZ.reshape(-1).tolist()))
    if key not in _CACHE:
        _CACHE[key] = _build_nc(plan, x.shape[1], W0.shape[1], W1.shape[1])
    nc = _CACHE[key]
    in_maps = _make_in_maps(x, W0, b0, W1, b1, plan, per_core)
    return nc, in_maps, plan


def kernel(x, edges, W0, b0, W1, b1):
    x = np.asarray(x)
    nc, in_maps, plan = build(x, edges, W0, b0, W1, b1)
    res = run_bass_kernel_spmd(nc, in_maps, list(range(plan.n_cores)))
    y = np.concatenate([r["y"] for r in res.results], axis=0)
    return y.astype(np.float32)
